# revision 15
# baseline (speedup 1.0000x reference)
"""GCN message-passing block on 8 Trainium2 NeuronCores.

Computes: delta = segment_sum((x @ W.T)[source] * edge_weights, target)

Strategy (edge-sharded, fully static SPMD program):
  By linearity, delta = segment_sum(x[source]*w, target) @ W.T -- the node
  projection commutes with the weighted aggregation, so W is applied AFTER
  aggregation (to ~100k rows) instead of per-edge (640k rows).

  Host side: each distinct target node gets a "compacted column". Columns
  are packed CPB=512 per PSUM bank; banks are distributed round-robin over
  the 8 cores. x is split into NCHUNK row-chunks so sources fit int16 for
  the hardware dma_gather. Within a bank, each (chunk c, stripe s) pair
  owns one gather tile of 128 slots; stripe s covers compact columns
  [64s, 64s+64). Edges overflowing their tile are deferred to later banks
  under fresh duplicate columns; the host adds duplicate rows at the end.

  Device side, per bank:
    1. NCHUNK dma_gathers fetch the source rows of x (512B each)
    2. DVE builds per-tile selectors S[e, col] = w_e * (tloc_e == col)
       via an iota-compare (batched over tiles)
    3. per tile: PE matmul Z[:, win] += X_tile.T @ S_tile accumulates the
       weighted segment sums for the bank's columns (dims on partitions)
    4. PE matmul out = Z_slice.T @ W.T flips orientation for free and
       applies the projection; result rows stream to DRAM contiguously.
"""

import numpy as np

import concourse.bacc as bacc
import concourse.bass as bass
import concourse.mybir as mybir
import concourse.tile as tile
from concourse.bass_utils import run_bass_kernel_spmd

N_CORES = 8
NUM_NODES = 100000
D = 128

NCHUNK = 4
CHUNK = NUM_NODES // NCHUNK   # 25000 rows per gather chunk (int16-addressable)
SWIDTH = 74      # columns per stripe == selector window width
NSTR = 7         # stripes per chunk (SWIDTH * NSTR >= CPB)
CPB = 512        # compacted columns per PSUM bank (one f32 bank)
SLOT = 128       # gather slots (edges) per tile
NB = 25          # banks per core
TPB = NCHUNK * NSTR            # tiles per bank (32)
SELBATCH = 8     # tiles per selector-build DVE op

NT = NB * TPB          # tiles per core
NCOL = NB * CPB        # output rows (compact columns) per core
NIDX = TPB * SLOT      # gather slots per bank (4096)
F32 = mybir.dt.float32
F16 = mybir.dt.float16
I16 = mybir.dt.int16


def _mk_ap(base, ap_list):
    return bass.AP(base.tensor, base.offset, ap_list)


def _bank_groups(nb, nbg):
    return [(g0, min(nbg, nb - g0)) for g0 in range(0, nb, nbg)]


def build_program(num_nodes=NUM_NODES, nb=NB, n_cores=N_CORES, stage_bufs=3,
                  repeat=1, do_gather=True, do_compute=True, n_queues=1,
                  single_packet=True, gsplit=1, nbg=1, psa_bufs=2,
                  psb_bufs=2, sel_bufs=3, zsb_bufs=2, osb_bufs=2, hw_loop=1):
    """Build + compile the single SPMD Bass program (data-independent).

    repeat>1 re-runs the whole pipeline unrolled; hw_loop>1 wraps the body
    in a hardware For_i loop (for slope-based benchmarking with one
    dispatch covering thousands of iterations).
    """
    import contextlib
    nt = nb * TPB
    ncol = nb * CPB
    chunk = num_nodes // NCHUNK
    nc = bacc.Bacc("TRN2", target_bir_lowering=False, debug=False,
                   num_devices=n_cores, num_swdge_queues=n_queues)
    x_t = nc.dram_tensor("x", [num_nodes, D], F16, kind="ExternalInput")
    wt_t = nc.dram_tensor("wt", [D, D], F16, kind="ExternalInput")
    # int16 gather indices: per (bank, chunk) a [128, SLOT*NSTR/16] block
    idx_t = nc.dram_tensor("idx16", [SLOT, nb * NCHUNK * (NSTR * SLOT // 16)],
                           I16, kind="ExternalInput")
    tloc_t = nc.dram_tensor("tloc", [SLOT, nt], F16, kind="ExternalInput")
    ew_t = nc.dram_tensor("ew", [SLOT, nt], F16, kind="ExternalInput")
    # materialized iota in [col][tile] layout: iota2[:, c*TPB+j] = c, so the
    # selector build's operands are all packed in the last free dim (the DVE
    # 2x_1p fast mode needs stride-1 last dims on every non-scalar operand)
    iota_t = nc.dram_tensor("iota2", [SLOT, SWIDTH * TPB], F16,
                            kind="ExternalInput")
    out_t = nc.dram_tensor("outc", [ncol, D], F32, kind="ExternalOutput")

    x_ap = x_t.ap()
    out_ap = out_t.ap()
    idxcols = NSTR * SLOT // 16   # 64 idx columns per (bank, chunk)

    with tile.TileContext(nc) as tc:
        with (
            tc.tile_pool(name="const", bufs=1) as constp,
            tc.tile_pool(name="stage", bufs=stage_bufs) as stagep,
            tc.tile_pool(name="sel", bufs=sel_bufs) as selp,
            tc.tile_pool(name="zsb", bufs=zsb_bufs) as zsbp,
            tc.tile_pool(name="outsb", bufs=osb_bufs) as outsbp,
            tc.tile_pool(name="psA", bufs=psa_bufs, space="PSUM") as psA,
            tc.tile_pool(name="psB", bufs=psb_bufs, space="PSUM") as psB,
        ):
            idx_sb = constp.tile([SLOT, nb * NCHUNK * idxcols], I16)
            tloc_sb = constp.tile([SLOT, nt], F16)
            ew_sb = constp.tile([SLOT, nt], F16)
            iota_sb = constp.tile([SLOT, SWIDTH * TPB], F16)
            wt_sb = constp.tile([D, D], F16)
            zconst = constp.tile([SLOT, CPB], F16)
            nc.vector.memset(zconst[:], 0.0)
            nc.sync.dma_start(idx_sb[:], idx_t.ap()[:])
            nc.sync.dma_start(tloc_sb[:], tloc_t.ap()[:])
            nc.sync.dma_start(ew_sb[:], ew_t.ap()[:])
            nc.sync.dma_start(iota_sb[:], iota_t.ap()[:])
            nc.sync.dma_start(wt_sb[:], wt_t.ap()[:])

            groups = _bank_groups(nb, nbg)
            gmax = max(gsz for _, gsz in groups)
            # one-time zero of the gather staging pool: slots whose gather
            # was trailing-trimmed keep stale buffer contents, which must be
            # finite (NaN * 0-selector = NaN in PSUM)
            for _z in range(stage_bufs):
                xz = stagep.tile([SLOT, gmax * TPB * D], F16, tag="xg")
                nc.vector.memset(xz[:], 0.0)
            loop_cm = (tc.For_i(0, hw_loop, 1) if hw_loop > 1
                       else contextlib.nullcontext())
            with loop_cm:
             for _rep in range(repeat):
              iblk = 0   # cumulative idx16 column offset (in idxcols units)
              for b0, gsz in groups:
                # 1) gather: per chunk, one dma_gather covering gsz banks.
                # xg free layout: [chunk][bank-in-group][stripe][D]
                xg = stagep.tile([SLOT, gmax * TPB * D], F16, tag="xg")
                gidx = gsz * NSTR * SLOT           # idx per gather
                for c in range(NCHUNK if do_gather else 0):
                    g0 = c * gsz * NSTR
                    oslice = xg[:, g0 * D:(g0 + gsz * NSTR) * D]
                    o3 = oslice.rearrange("p (g e) -> p g e", e=D)
                    i0 = (iblk + c * gsz) * idxcols
                    nc.gpsimd.dma_gather(
                        out_ap=o3,
                        in_ap=x_ap[c * chunk:(c + 1) * chunk, :],
                        idxs_ap=idx_sb[:, i0:i0 + gidx // 16],
                        num_idxs=gidx,
                        num_idxs_reg=gidx,
                        elem_size=D,
                        queue_num=c % n_queues,
                        single_packet=single_packet,
                    )
                iblk += gsz * NCHUNK

                # 2) selector build: S[e, col, j] = ew * (tloc == col), in a
                # [col][tile] layout so every operand's last free dim is
                # packed (tile index, stride 1) -> DVE 2x_1p fast mode.
                if not do_compute:
                    continue
                for bg in range(gsz):
                  b = b0 + bg
                  t0 = b * TPB
                  S = selp.tile([SLOT, SWIDTH * TPB], F16, tag="sel")
                  s3 = S[:].rearrange("p (w g) -> p w g", g=TPB)
                  io3 = iota_sb[:].rearrange("p (w g) -> p w g", g=TPB)
                  tl = tloc_sb[:, t0:t0 + TPB]
                  tl_b = _mk_ap(tl, tl.ap[:1] + [[0, SWIDTH]] + tl.ap[1:])
                  ew = ew_sb[:, t0:t0 + TPB]
                  ew_b = _mk_ap(ew, ew.ap[:1] + [[0, SWIDTH]] + ew.ap[1:])
                  nc.vector.tensor_tensor(
                      out=s3, in0=io3, in1=tl_b,
                      op=mybir.AluOpType.is_equal)
                  nc.vector.tensor_tensor(
                      out=s3, in0=s3, in1=ew_b,
                      op=mybir.AluOpType.mult)

                  # 3) accumulate weighted segment sums into the PSUM bank.
                  # A full-width zeroing matmul opens the group: start=True
                  # marks the whole 2KB PSUM zero-region pending-zero, so
                  # the zero write must cover the full bank before the
                  # windowed accumulates (cheaper on PE than a DVE memset).
                  zp = psA.tile([SLOT, CPB], F32, tag="zp")
                  nc.tensor.matmul(
                      out=zp[:], lhsT=wt_sb[:], rhs=zconst[:],
                      start=True, stop=False, skip_group_check=True,
                  )
                  for j in range(TPB):
                      w0 = SWIDTH * (j % NSTR)
                      wd = min(SWIDTH, CPB - w0)
                      jc, js = j // NSTR, j % NSTR
                      xslice = (jc * gsz + bg) * NSTR + js
                      rv = S[:, j:j + 1]
                      rhs = _mk_ap(rv, [rv.ap[0], [TPB, wd]])
                      nc.tensor.matmul(
                          out=zp[:, w0:w0 + wd],
                          lhsT=xg[:, xslice * D:(xslice + 1) * D],
                          rhs=rhs,
                          start=False, stop=(j == TPB - 1),
                          skip_group_check=True,
                      )

                  # 4) apply W.T: out rows (targets) = Z_slice.T @ W.T
                  zsb = zsbp.tile([SLOT, CPB], F16, tag="zsb")
                  nc.scalar.copy(zsb[:], zp[:])
                  ob = psB.tile([SLOT, CPB], F32, tag="ob")
                  for q in range(CPB // D):
                      nc.tensor.matmul(
                          out=ob[:, q * D:(q + 1) * D],
                          lhsT=zsb[:, q * D:(q + 1) * D],
                          rhs=wt_sb[:],
                          start=True, stop=True,
                      )
                  osb = outsbp.tile([SLOT, CPB], F32, tag="osb")
                  nc.scalar.copy(osb[:], ob[:])
                  dro = out_ap[b * CPB:(b + 1) * CPB, :].rearrange(
                      "(q p) d -> p q d", p=SLOT)
                  sro = osb[:].rearrange("p (q d) -> p q d", d=D)
                  nc.sync.dma_start(dro, sro)

    nc.compile()
    return nc


_PROGRAM_CACHE = {}

# tuned configuration (see bench history): 4 SWDGE queues so all four Q7
# core-pairs generate gather descriptors in parallel; multi-packet gathers;
# 6 staging buffers so many gathers stay in flight.
TUNED = dict(n_queues=4, single_packet=False, nbg=1, stage_bufs=6)


def _get_program(key="full", **kw):
    if key not in _PROGRAM_CACHE:
        _PROGRAM_CACHE[key] = build_program(**kw)
    return _PROGRAM_CACHE[key]


def preprocess(source, target, edge_weights, num_nodes=NUM_NODES, nb=NB,
               n_cores=N_CORES, nbg=1):
    """Assign edges to (core, bank, chunk, stripe, slot), targets to columns.

    Returns idx16 (replicated int16 gather indices), tloc, ew arrays, the
    column->target map, and leftover edges exceeding capacity (host handles;
    expected empty).
    """
    chunk = num_nodes // NCHUNK
    nt = nb * TPB
    n_banks = nb * n_cores
    idxcols = NSTR * SLOT // 16

    order = np.argsort(target, kind="stable")
    r_src = source[order].astype(np.int64)
    r_tgt = target[order].astype(np.int64)
    r_w = edge_weights[order].astype(np.float32)

    # idx stream per (core, bank, chunk): int16[NSTR*SLOT]; pad entries
    # spread across rows (same-row hammering serializes on one HBM row)
    pad = (np.arange(NSTR * SLOT, dtype=np.int64) * 97) % chunk
    idxs = np.broadcast_to(pad.astype(np.int16),
                           (n_cores, nb * NCHUNK, NSTR * SLOT)).copy()
    tloc = np.full((n_cores, SLOT, nt), -1.0, np.float32)
    ewa = np.zeros((n_cores, SLOT, nt), np.float32)
    colmap = np.full((n_cores, nb * CPB), -1, np.int64)

    gb = 0
    leftover = (np.zeros(0, np.int64), np.zeros(0, np.int64),
                np.zeros(0, np.float32))

    while r_tgt.size and gb < n_banks:
        ut, ucnt = np.unique(r_tgt, return_counts=True)
        n_u = ut.size
        ucol = 0
        ecur = 0
        defer = []
        while ucol < n_u and gb < n_banks:
            core = gb % n_cores
            bl = gb // n_cores
            take_u = min(CPB, n_u - ucol)
            bank_ut = ut[ucol:ucol + take_u]
            bank_cnt = ucnt[ucol:ucol + take_u]
            colmap[core, bl * CPB:bl * CPB + take_u] = bank_ut
            e_end = ecur + int(bank_cnt.sum())
            ecol = np.repeat(np.arange(take_u, dtype=np.int64), bank_cnt)
            b_src = r_src[ecur:e_end]
            b_tgt = r_tgt[ecur:e_end]
            b_w = r_w[ecur:e_end]
            b_chunk = b_src // chunk
            b_stripe = ecol // SWIDTH
            # order edges by (chunk, stripe) for grouped slot assignment
            o2 = np.lexsort((b_stripe, b_chunk))
            b_src, b_tgt, b_w = b_src[o2], b_tgt[o2], b_w[o2]
            ecol, b_chunk, b_stripe = ecol[o2], b_chunk[o2], b_stripe[o2]
            key = b_chunk * NSTR + b_stripe
            starts = np.searchsorted(key, np.arange(NCHUNK * NSTR + 1))
            for cs in range(NCHUNK * NSTR):
                lo, hi = int(starts[cs]), int(starts[cs + 1])
                n_e = hi - lo
                if n_e == 0:
                    continue
                c, s = cs // NSTR, cs % NSTR
                k = min(n_e, SLOT)
                sl = slice(lo, lo + k)
                ct = bl * TPB + c * NSTR + s          # tile index in core
                slots = np.arange(k)
                idxs[core, bl * NCHUNK + c, s * SLOT:s * SLOT + k] = (
                    b_src[sl] - c * chunk).astype(np.int16)
                tloc[core, slots, ct] = (ecol[sl] - SWIDTH * s
                                         ).astype(np.float32)
                ewa[core, slots, ct] = b_w[sl]
                if k < n_e:
                    dsl = slice(lo + k, hi)
                    defer.append((b_src[dsl], b_tgt[dsl], b_w[dsl]))
            ucol += take_u
            ecur = e_end
            gb += 1
        if ucol < n_u:
            defer.append((r_src[ecur:], r_tgt[ecur:], r_w[ecur:]))
        if defer:
            r_src = np.concatenate([d[0] for d in defer])
            r_tgt = np.concatenate([d[1] for d in defer])
            r_w = np.concatenate([d[2] for d in defer])
            o3 = np.argsort(r_tgt, kind="stable")
            r_src, r_tgt, r_w = r_src[o3], r_tgt[o3], r_w[o3]
        else:
            r_src = r_tgt = np.zeros(0, np.int64)
            r_w = np.zeros(0, np.float32)
    if r_tgt.size:
        leftover = (r_src, r_tgt, r_w)

    # trailing-pad trim: the gather ucode drops trailing negative indices
    # from each stream, so mark every stream's trailing pad slots with -1.
    # (Mid-stream pads must stay >= 0: they are really gathered.)
    if nbg == 1:
        real = (tloc >= 0).transpose(0, 2, 1).reshape(
            n_cores, nb * NCHUNK, NSTR * SLOT)
        has = real.any(axis=2)
        lastpos = (NSTR * SLOT - 1) - np.argmax(real[:, :, ::-1], axis=2)
        cut = np.where(has, lastpos + 1, 0)
        pos = np.arange(NSTR * SLOT)
        idxs[pos[None, None, :] >= cut[:, :, None]] = -1

    # regroup streams: one gather block per (bank-group, chunk); wrap into
    # the [128, .../16] int16 layout (pos i -> [i%16, i//16]), 8x replicated
    idx16 = np.zeros((n_cores, SLOT, nb * NCHUNK * idxcols), np.int16)
    col = 0
    strm = idxs.reshape(n_cores, nb, NCHUNK, NSTR * SLOT)
    for b0, gsz in _bank_groups(nb, nbg):
        for c in range(NCHUNK):
            blk = strm[:, b0:b0 + gsz, c, :].reshape(n_cores, -1)
            w = blk.shape[1] // 16
            st = blk.reshape(n_cores, w, 16).transpose(0, 2, 1)
            for k in range(8):
                idx16[:, 16 * k:16 * (k + 1), col:col + w] = st
            col += w
    return idx16, tloc, ewa, colmap, leftover


def build_in_maps(x, W, edge_weights, src, tgt, tuned):
    """Host preprocessing -> per-core input maps (shared w/ bench_final)."""
    x16 = np.ascontiguousarray(np.asarray(x, np.float32).astype(np.float16))
    idx16, tloc, ewa, colmap, leftover = preprocess(
        src, tgt, np.asarray(edge_weights, np.float32), nbg=tuned["nbg"])
    wt = np.ascontiguousarray(np.asarray(W, np.float32).T).astype(np.float16)
    iota2 = np.broadcast_to(
        np.repeat(np.arange(SWIDTH, dtype=np.float16), TPB),
        (SLOT, SWIDTH * TPB)).copy()
    in_maps = [
        {"x": x16, "wt": wt, "idx16": idx16[c],
         "tloc": tloc[c].astype(np.float16), "ew": ewa[c].astype(np.float16),
         "iota2": iota2}
        for c in range(N_CORES)
    ]
    build_in_maps.aux = (colmap, leftover)
    return in_maps


def kernel(x, W, edge_weights, source, target):
    x = np.ascontiguousarray(np.asarray(x, np.float32))
    W = np.asarray(W, np.float32)
    edge_weights = np.asarray(edge_weights, np.float32)
    src = np.asarray(source).astype(np.int64)
    tgt = np.asarray(target).astype(np.int64)
    num_nodes, d = x.shape
    assert d == D and num_nodes == NUM_NODES, (x.shape,)

    in_maps = build_in_maps(x, W, edge_weights, src, tgt, TUNED)
    colmap, leftover = build_in_maps.aux

    nc = _get_program("full", **TUNED)
    res = run_bass_kernel_spmd(nc, in_maps, core_ids=list(range(N_CORES)))

    out = np.zeros((num_nodes, D), np.float32)
    all_rows = np.concatenate([res.results[c]["outc"] for c in range(N_CORES)])
    all_cols = colmap.reshape(-1)
    valid = all_cols >= 0
    t_ids = all_cols[valid]
    rows = all_rows[valid]
    uniq, first = np.unique(t_ids, return_index=True)
    out[t_ids[first]] = rows[first]
    dup = np.ones(t_ids.size, bool)
    dup[first] = False
    if dup.any():
        np.add.at(out, t_ids[dup], rows[dup])
    l_src, l_tgt, l_w = leftover
    if l_tgt.size:
        np.add.at(out, l_tgt, (x[l_src] * l_w[:, None]) @ W.T)
    return out



# revision 23
# speedup vs baseline: 1.1060x; 1.1060x over previous
"""GCN message-passing block on 8 Trainium2 NeuronCores.

Computes: delta = segment_sum((x @ W.T)[source] * edge_weights, target)

Strategy (edge-sharded, fully static SPMD program):
  By linearity, delta = segment_sum(x[source]*w, target) @ W.T -- the node
  projection commutes with the weighted aggregation, so W is applied AFTER
  aggregation (to ~100k rows) instead of per-edge (640k rows).

  Host side: each distinct target node gets a "compacted column". Columns
  are packed CPB=512 per PSUM bank; banks are distributed round-robin over
  the 8 cores. x is split into NCHUNK row-chunks so sources fit int16 for
  the hardware dma_gather. Within a bank, each (chunk c, stripe s) pair
  owns one gather tile of 128 slots; stripe s covers compact columns
  [64s, 64s+64). Edges overflowing their tile are deferred to later banks
  under fresh duplicate columns; the host adds duplicate rows at the end.

  Device side, per bank:
    1. NCHUNK dma_gathers fetch the source rows of x (512B each)
    2. DVE builds per-tile selectors S[e, col] = w_e * (tloc_e == col)
       via an iota-compare (batched over tiles)
    3. per tile: PE matmul Z[:, win] += X_tile.T @ S_tile accumulates the
       weighted segment sums for the bank's columns (dims on partitions)
    4. PE matmul out = Z_slice.T @ W.T flips orientation for free and
       applies the projection; result rows stream to DRAM contiguously.
"""

import numpy as np

import concourse.bacc as bacc
import concourse.bass as bass
import concourse.mybir as mybir
import concourse.tile as tile
from concourse.bass_utils import run_bass_kernel_spmd

N_CORES = 8
NUM_NODES = 100000
D = 128

NCHUNK = 2
CHUNK = NUM_NODES // NCHUNK   # 50000 rows per gather chunk
# The gather ucode sign-extends int16 indices, so biasing the gather base
# +32768 rows into the chunk gives +/-32768 reach = 65536 rows >= 50000.
# Only trailing-negative stream indices are dropped by the ucode, so the
# host guarantees each stream's final index is >= 0 (tail swap).
BIAS = 32768
SWIDTH = 37      # columns per stripe == selector window width
NSTR = 14        # stripes per chunk (SWIDTH * NSTR >= CPB)
CPB = 512        # compacted columns per PSUM bank (one f32 bank)
SLOT = 128       # gather slots (edges) per tile
NB = 25          # banks per core
TPB = NCHUNK * NSTR            # tiles per bank (32)
SELBATCH = 8     # tiles per selector-build DVE op
TRIM = False     # trailing-trim measured slower (ring-space mismatch)

NT = NB * TPB          # tiles per core
NCOL = NB * CPB        # output rows (compact columns) per core
NIDX = TPB * SLOT      # gather slots per bank (4096)
F32 = mybir.dt.float32
F16 = mybir.dt.float16
I16 = mybir.dt.int16


def _mk_ap(base, ap_list):
    return bass.AP(base.tensor, base.offset, ap_list)


def _bank_groups(nb, nbg):
    return [(g0, min(nbg, nb - g0)) for g0 in range(0, nb, nbg)]


def build_program(num_nodes=NUM_NODES, nb=NB, n_cores=N_CORES, stage_bufs=3,
                  repeat=1, do_gather=True, do_compute=True, n_queues=1,
                  single_packet=True, gsplit=1, nbg=1, psa_bufs=2,
                  psb_bufs=2, sel_bufs=3, zsb_bufs=2, osb_bufs=2, hw_loop=1):
    """Build + compile the single SPMD Bass program (data-independent).

    repeat>1 re-runs the whole pipeline unrolled; hw_loop>1 wraps the body
    in a hardware For_i loop (for slope-based benchmarking with one
    dispatch covering thousands of iterations).
    """
    import contextlib
    nt = nb * TPB
    ncol = nb * CPB
    chunk = num_nodes // NCHUNK
    nc = bacc.Bacc("TRN2", target_bir_lowering=False, debug=False,
                   num_devices=n_cores, num_swdge_queues=n_queues)
    x_t = nc.dram_tensor("x", [num_nodes, D], F16, kind="ExternalInput")
    wt_t = nc.dram_tensor("wt", [D, D], F16, kind="ExternalInput")
    # int16 gather indices: per (bank, chunk) a [128, SLOT*NSTR/16] block
    idx_t = nc.dram_tensor("idx16", [SLOT, nb * NCHUNK * (NSTR * SLOT // 16)],
                           I16, kind="ExternalInput")
    tloc_t = nc.dram_tensor("tloc", [SLOT, nt], F16, kind="ExternalInput")
    ew_t = nc.dram_tensor("ew", [SLOT, nt], F16, kind="ExternalInput")
    # materialized iota in [col][tile] layout: iota2[:, c*TPB+j] = c, so the
    # selector build's operands are all packed in the last free dim (the DVE
    # 2x_1p fast mode needs stride-1 last dims on every non-scalar operand)
    iota_t = nc.dram_tensor("iota2", [SLOT, SWIDTH * TPB], F16,
                            kind="ExternalInput")
    out_t = nc.dram_tensor("outc", [ncol, D], F32, kind="ExternalOutput")

    x_ap = x_t.ap()
    out_ap = out_t.ap()
    idxcols = NSTR * SLOT // 16   # 64 idx columns per (bank, chunk)

    with tile.TileContext(nc) as tc:
        with (
            tc.tile_pool(name="const", bufs=1) as constp,
            tc.tile_pool(name="stage", bufs=stage_bufs) as stagep,
            tc.tile_pool(name="sel", bufs=sel_bufs) as selp,
            tc.tile_pool(name="zsb", bufs=zsb_bufs) as zsbp,
            tc.tile_pool(name="outsb", bufs=osb_bufs) as outsbp,
            tc.tile_pool(name="psA", bufs=psa_bufs, space="PSUM") as psA,
            tc.tile_pool(name="psB", bufs=psb_bufs, space="PSUM") as psB,
        ):
            idx_sb = constp.tile([SLOT, nb * NCHUNK * idxcols], I16)
            tloc_sb = constp.tile([SLOT, nt], F16)
            ew_sb = constp.tile([SLOT, nt], F16)
            iota_sb = constp.tile([SLOT, SWIDTH * TPB], F16)
            wt_sb = constp.tile([D, D], F16)
            zconst = constp.tile([SLOT, CPB], F16)
            nc.vector.memset(zconst[:], 0.0)
            nc.sync.dma_start(idx_sb[:], idx_t.ap()[:])
            nc.sync.dma_start(tloc_sb[:], tloc_t.ap()[:])
            nc.sync.dma_start(ew_sb[:], ew_t.ap()[:])
            nc.sync.dma_start(iota_sb[:], iota_t.ap()[:])
            nc.sync.dma_start(wt_sb[:], wt_t.ap()[:])

            groups = _bank_groups(nb, nbg)
            gmax = max(gsz for _, gsz in groups)
            loop_cm = (tc.For_i(0, hw_loop, 1) if hw_loop > 1
                       else contextlib.nullcontext())
            with loop_cm:
             for _rep in range(repeat):
              iblk = 0   # cumulative idx16 column offset (in idxcols units)
              for b0, gsz in groups:
                # 1) gather: per chunk, one dma_gather covering gsz banks.
                # xg free layout: [chunk][bank-in-group][stripe][D]
                xg = stagep.tile([SLOT, gmax * TPB * D], F16, tag="xg")
                gidx = gsz * NSTR * SLOT           # idx per gather
                for c in range(NCHUNK if do_gather else 0):
                    g0 = c * gsz * NSTR
                    oslice = xg[:, g0 * D:(g0 + gsz * NSTR) * D]
                    o3 = oslice.rearrange("p (g e) -> p g e", e=D)
                    i0 = (iblk + c * gsz) * idxcols
                    nc.gpsimd.dma_gather(
                        out_ap=o3,
                        in_ap=x_ap[c * chunk + BIAS:(c + 1) * chunk, :],
                        idxs_ap=idx_sb[:, i0:i0 + gidx // 16],
                        num_idxs=gidx,
                        num_idxs_reg=gidx,
                        elem_size=D,
                        queue_num=(b0 * NCHUNK + c) % n_queues,
                        single_packet=single_packet,
                    )
                iblk += gsz * NCHUNK

                # 2) selector build: S[e, col, j] = ew * (tloc == col), in a
                # [col][tile] layout so every operand's last free dim is
                # packed (tile index, stride 1) -> DVE 2x_1p fast mode.
                if not do_compute:
                    continue
                for bg in range(gsz):
                  b = b0 + bg
                  t0 = b * TPB
                  S = selp.tile([SLOT, SWIDTH * TPB], F16, tag="sel")
                  s3 = S[:].rearrange("p (w g) -> p w g", g=TPB)
                  io3 = iota_sb[:].rearrange("p (w g) -> p w g", g=TPB)
                  tl = tloc_sb[:, t0:t0 + TPB]
                  tl_b = _mk_ap(tl, tl.ap[:1] + [[0, SWIDTH]] + tl.ap[1:])
                  ew = ew_sb[:, t0:t0 + TPB]
                  ew_b = _mk_ap(ew, ew.ap[:1] + [[0, SWIDTH]] + ew.ap[1:])
                  nc.vector.tensor_tensor(
                      out=s3, in0=io3, in1=tl_b,
                      op=mybir.AluOpType.is_equal)
                  nc.vector.tensor_tensor(
                      out=s3, in0=s3, in1=ew_b,
                      op=mybir.AluOpType.mult)

                  # 3) accumulate weighted segment sums into the PSUM bank.
                  # A full-width zeroing matmul opens the group: start=True
                  # marks the whole 2KB PSUM zero-region pending-zero, so
                  # the zero write must cover the full bank before the
                  # windowed accumulates (cheaper on PE than a DVE memset).
                  zp = psA.tile([SLOT, CPB], F32, tag="zp")
                  nc.tensor.matmul(
                      out=zp[:], lhsT=wt_sb[:], rhs=zconst[:],
                      start=True, stop=False, skip_group_check=True,
                  )
                  for j in range(TPB):
                      w0 = SWIDTH * (j % NSTR)
                      wd = min(SWIDTH, CPB - w0)
                      jc, js = j // NSTR, j % NSTR
                      xslice = (jc * gsz + bg) * NSTR + js
                      rv = S[:, j:j + 1]
                      rhs = _mk_ap(rv, [rv.ap[0], [TPB, wd]])
                      nc.tensor.matmul(
                          out=zp[:, w0:w0 + wd],
                          lhsT=xg[:, xslice * D:(xslice + 1) * D],
                          rhs=rhs,
                          start=False, stop=(j == TPB - 1),
                          skip_group_check=True,
                      )

                  # 4) apply W.T: out rows (targets) = Z_slice.T @ W.T
                  zsb = zsbp.tile([SLOT, CPB], F16, tag="zsb")
                  nc.scalar.copy(zsb[:], zp[:])
                  ob = psB.tile([SLOT, CPB], F32, tag="ob")
                  for q in range(CPB // D):
                      nc.tensor.matmul(
                          out=ob[:, q * D:(q + 1) * D],
                          lhsT=zsb[:, q * D:(q + 1) * D],
                          rhs=wt_sb[:],
                          start=True, stop=True,
                      )
                  osb = outsbp.tile([SLOT, CPB], F32, tag="osb")
                  nc.scalar.copy(osb[:], ob[:])
                  dro = out_ap[b * CPB:(b + 1) * CPB, :].rearrange(
                      "(q p) d -> p q d", p=SLOT)
                  sro = osb[:].rearrange("p (q d) -> p q d", d=D)
                  nc.sync.dma_start(dro, sro)

    nc.compile()
    return nc


_PROGRAM_CACHE = {}

# tuned configuration (see bench history): 4 SWDGE queues so all four Q7
# core-pairs generate gather descriptors in parallel; multi-packet gathers;
# 6 staging buffers so many gathers stay in flight.
TUNED = dict(n_queues=4, single_packet=False, nbg=1, stage_bufs=6)


def _get_program(key="full", **kw):
    if key not in _PROGRAM_CACHE:
        _PROGRAM_CACHE[key] = build_program(**kw)
    return _PROGRAM_CACHE[key]


def preprocess(source, target, edge_weights, num_nodes=NUM_NODES, nb=NB,
               n_cores=N_CORES, nbg=1):
    """Assign edges to (core, bank, chunk, stripe, slot), targets to columns.

    Returns idx16 (replicated int16 gather indices), tloc, ew arrays, the
    column->target map, and leftover edges exceeding capacity (host handles;
    expected empty).
    """
    chunk = num_nodes // NCHUNK
    nt = nb * TPB
    n_banks = nb * n_cores
    idxcols = NSTR * SLOT // 16

    order = np.argsort(target, kind="stable")
    r_src = source[order].astype(np.int64)
    r_tgt = target[order].astype(np.int64)
    r_w = edge_weights[order].astype(np.float32)

    # idx stream per (core, bank, chunk): int16[NSTR*SLOT]; pad entries
    # spread across rows (same-row hammering serializes on one HBM row).
    # Pads must be >= 0 (trailing negatives would be dropped by the ucode);
    # positive int16 reach above the biased base is BIAS - 1 - (chunk-BIAS).
    pad = (np.arange(NSTR * SLOT, dtype=np.int64) * 97) % min(chunk, 17000)
    idxs = np.broadcast_to(pad.astype(np.int16),
                           (n_cores, nb * NCHUNK, NSTR * SLOT)).copy()
    tloc = np.full((n_cores, SLOT, nt), -1.0, np.float32)
    ewa = np.zeros((n_cores, SLOT, nt), np.float32)
    colmap = np.full((n_cores, nb * CPB), -1, np.int64)

    gb = 0
    leftover = (np.zeros(0, np.int64), np.zeros(0, np.int64),
                np.zeros(0, np.float32))

    while r_tgt.size and gb < n_banks:
        ut, ucnt = np.unique(r_tgt, return_counts=True)
        n_u = ut.size
        ucol = 0
        ecur = 0
        defer = []
        while ucol < n_u and gb < n_banks:
            core = gb % n_cores
            bl = gb // n_cores
            take_u = min(CPB, n_u - ucol)
            bank_ut = ut[ucol:ucol + take_u]
            bank_cnt = ucnt[ucol:ucol + take_u]
            colmap[core, bl * CPB:bl * CPB + take_u] = bank_ut
            e_end = ecur + int(bank_cnt.sum())
            ecol = np.repeat(np.arange(take_u, dtype=np.int64), bank_cnt)
            b_src = r_src[ecur:e_end]
            b_tgt = r_tgt[ecur:e_end]
            b_w = r_w[ecur:e_end]
            b_chunk = b_src // chunk
            b_stripe = ecol // SWIDTH
            # order edges by (chunk, stripe) for grouped slot assignment
            o2 = np.lexsort((b_stripe, b_chunk))
            b_src, b_tgt, b_w = b_src[o2], b_tgt[o2], b_w[o2]
            ecol, b_chunk, b_stripe = ecol[o2], b_chunk[o2], b_stripe[o2]
            key = b_chunk * NSTR + b_stripe
            starts = np.searchsorted(key, np.arange(NCHUNK * NSTR + 1))
            for cs in range(NCHUNK * NSTR):
                lo, hi = int(starts[cs]), int(starts[cs + 1])
                n_e = hi - lo
                if n_e == 0:
                    continue
                c, s = cs // NSTR, cs % NSTR
                k = min(n_e, SLOT)
                sl = slice(lo, lo + k)
                ct = bl * TPB + c * NSTR + s          # tile index in core
                slots = np.arange(k)
                idxs[core, bl * NCHUNK + c, s * SLOT:s * SLOT + k] = (
                    b_src[sl] - c * chunk - BIAS).astype(np.int16)
                tloc[core, slots, ct] = (ecol[sl] - SWIDTH * s
                                         ).astype(np.float32)
                ewa[core, slots, ct] = b_w[sl]
                if k < n_e:
                    dsl = slice(lo + k, hi)
                    defer.append((b_src[dsl], b_tgt[dsl], b_w[dsl]))
            ucol += take_u
            ecur = e_end
            gb += 1
        if ucol < n_u:
            defer.append((r_src[ecur:], r_tgt[ecur:], r_w[ecur:]))
        if defer:
            r_src = np.concatenate([d[0] for d in defer])
            r_tgt = np.concatenate([d[1] for d in defer])
            r_w = np.concatenate([d[2] for d in defer])
            o3 = np.argsort(r_tgt, kind="stable")
            r_src, r_tgt, r_w = r_src[o3], r_tgt[o3], r_w[o3]
        else:
            r_src = r_tgt = np.zeros(0, np.int64)
            r_w = np.zeros(0, np.float32)
    if r_tgt.size:
        leftover = (r_src, r_tgt, r_w)

    # tail swap: the gather ucode drops trailing negative indices, so a
    # stream must never END with a biased-negative real edge.  Swap the
    # final slot with a non-negative-index slot of the same tile (slot
    # order within a tile is free); pads are always >= 0.
    lt_extra = []
    for core in range(n_cores):
        for bl in range(nb):
            for c in range(NCHUNK):
                st = idxs[core, bl * NCHUNK + c]
                if st[-1] >= 0:
                    continue
                ct = bl * TPB + c * NSTR + (NSTR - 1)
                base = (NSTR - 1) * SLOT
                cand = np.nonzero(st[base:-1] >= 0)[0]
                if cand.size:
                    j = base + int(cand[-1])
                    st[j], st[-1] = int(st[-1]), int(st[j])
                    for arr in (tloc, ewa):
                        a, b = arr[core, j - base, ct], arr[core, SLOT - 1, ct]
                        arr[core, j - base, ct] = b
                        arr[core, SLOT - 1, ct] = a
                else:
                    # whole last tile is negative-index real edges (never
                    # seen in practice): demote the final edge to the host
                    e_src = int(st[-1]) + c * chunk + BIAS
                    e_col = int(tloc[core, SLOT - 1, ct])
                    e_tgt = colmap[core, bl * CPB + (NSTR - 1) * SWIDTH + e_col]
                    lt_extra.append((e_src, int(e_tgt),
                                     float(ewa[core, SLOT - 1, ct])))
                    st[-1] = 0
                    tloc[core, SLOT - 1, ct] = -1.0
                    ewa[core, SLOT - 1, ct] = 0.0
    if lt_extra:
        ls, lt, lw = leftover
        leftover = (np.concatenate([ls, np.array([e[0] for e in lt_extra])]),
                    np.concatenate([lt, np.array([e[1] for e in lt_extra])]),
                    np.concatenate([lw, np.array([e[2] for e in lt_extra],
                                                 np.float32)]))

    # trailing-pad trim: the gather ucode drops trailing negative indices
    # from each stream, so mark every stream's trailing pad slots with -1.
    # (Mid-stream pads must stay >= 0: they are really gathered.)
    if nbg == 1 and TRIM:
        real = (tloc >= 0).transpose(0, 2, 1).reshape(
            n_cores, nb * NCHUNK, NSTR * SLOT)
        has = real.any(axis=2)
        lastpos = (NSTR * SLOT - 1) - np.argmax(real[:, :, ::-1], axis=2)
        cut = np.where(has, lastpos + 1, 0)
        pos = np.arange(NSTR * SLOT)
        idxs[pos[None, None, :] >= cut[:, :, None]] = -1

    # regroup streams: one gather block per (bank-group, chunk); wrap into
    # the [128, .../16] int16 layout (pos i -> [i%16, i//16]), 8x replicated
    idx16 = np.zeros((n_cores, SLOT, nb * NCHUNK * idxcols), np.int16)
    col = 0
    strm = idxs.reshape(n_cores, nb, NCHUNK, NSTR * SLOT)
    for b0, gsz in _bank_groups(nb, nbg):
        for c in range(NCHUNK):
            blk = strm[:, b0:b0 + gsz, c, :].reshape(n_cores, -1)
            w = blk.shape[1] // 16
            st = blk.reshape(n_cores, w, 16).transpose(0, 2, 1)
            for k in range(8):
                idx16[:, 16 * k:16 * (k + 1), col:col + w] = st
            col += w
    return idx16, tloc, ewa, colmap, leftover


def build_in_maps(x, W, edge_weights, src, tgt, tuned):
    """Host preprocessing -> per-core input maps (shared w/ bench_final)."""
    x16 = np.ascontiguousarray(np.asarray(x, np.float32).astype(np.float16))
    idx16, tloc, ewa, colmap, leftover = preprocess(
        src, tgt, np.asarray(edge_weights, np.float32), nbg=tuned["nbg"])
    wt = np.ascontiguousarray(np.asarray(W, np.float32).T).astype(np.float16)
    iota2 = np.broadcast_to(
        np.repeat(np.arange(SWIDTH, dtype=np.float16), TPB),
        (SLOT, SWIDTH * TPB)).copy()
    in_maps = [
        {"x": x16, "wt": wt, "idx16": idx16[c],
         "tloc": tloc[c].astype(np.float16), "ew": ewa[c].astype(np.float16),
         "iota2": iota2}
        for c in range(N_CORES)
    ]
    build_in_maps.aux = (colmap, leftover)
    return in_maps


def kernel(x, W, edge_weights, source, target):
    x = np.ascontiguousarray(np.asarray(x, np.float32))
    W = np.asarray(W, np.float32)
    edge_weights = np.asarray(edge_weights, np.float32)
    src = np.asarray(source).astype(np.int64)
    tgt = np.asarray(target).astype(np.int64)
    num_nodes, d = x.shape
    assert d == D and num_nodes == NUM_NODES, (x.shape,)

    in_maps = build_in_maps(x, W, edge_weights, src, tgt, TUNED)
    colmap, leftover = build_in_maps.aux

    nc = _get_program("full", **TUNED)
    res = run_bass_kernel_spmd(nc, in_maps, core_ids=list(range(N_CORES)))

    out = np.zeros((num_nodes, D), np.float32)
    all_rows = np.concatenate([res.results[c]["outc"] for c in range(N_CORES)])
    all_cols = colmap.reshape(-1)
    valid = all_cols >= 0
    t_ids = all_cols[valid]
    rows = all_rows[valid]
    uniq, first = np.unique(t_ids, return_index=True)
    out[t_ids[first]] = rows[first]
    dup = np.ones(t_ids.size, bool)
    dup[first] = False
    if dup.any():
        np.add.at(out, t_ids[dup], rows[dup])
    l_src, l_tgt, l_w = leftover
    if l_tgt.size:
        np.add.at(out, l_tgt, (x[l_src] * l_w[:, None]) @ W.T)
    return out

